# revision 1
# baseline (speedup 1.0000x reference)
"""CenterLoss kernel for Trainium2 (8 NeuronCores, SPMD data-parallel).

Math (per reference):
    c_i   = centers[labels[i]]                  # gather, (B, D)
    d_i   = ||x_i||^2 + ||c_i||^2 - 2 x_i.c_i   # == ||x_i - c_i||^2
    out   = mean(clip(d_i, 1e-12, 1e12))

Strategy (target_regime=memory):
  - Shard the batch (4096) across 8 cores -> 512 samples/core.
  - Replicate centers (100MB) into each core's DRAM, but only *read* the
    512 needed rows per core via indirect (gather) DMA -> ~2MB of HBM
    traffic per core instead of streaming all 100MB of centers.
  - p-major layout: partition p handles samples 4p..4p+3, so the x and
    labels shards load as single fully-contiguous DMAs
    (x_shard.reshape(128, 2048), labels_shard.reshape(128, 4)).
  - Raw Bacc (no Tile) with hand-placed semaphores:
      SP   : labels DMA (HWDGE) first — it gates the gathers
      Pool : x DMA (SWDGE), then 4 indirect row-gathers (one per
             128-sample chunk, each with its own completion sem since
             SWDGE completions can be out of order)
      DVE  : d_k = x_k - c_k per chunk as its gather lands
      ACT  : chunks 0..2: Square + free-axis accumulate into res column
      DVE  : last chunk squared+reduced on DVE (avoids ACT's serial tail)
      SP   : out DMA; completion sem tracked, epilogue drain enforces it
  - Each core returns its 512 per-sample distances; host does the final
    clip + mean (the unshard/all-reduce step).
"""

import os

import numpy as np

import concourse.bacc as bacc
import concourse.bass as bass
import concourse.mybir as mybir
from concourse.bass_utils import run_bass_kernel_spmd

N_CORES = 8
BATCH = 4096
FEAT = 512
NUM_CLASSES = 50000
SHARD = BATCH // N_CORES  # 512 samples per core
P = 128
N_CHUNKS = SHARD // P  # 4 samples per partition

CLAMP_MIN = 1e-12
CLAMP_MAX = 1e12

_cached_nc = None

# Last BassKernelResults (for test harnesses that want exec_time_ns).
LAST_RESULT = None


def _build_nc():
    nc = bacc.Bacc("TRN2", target_bir_lowering=False, debug=False, num_swdge_queues=2)

    # x is fed pre-reshaped to [128, 4*512]: partition p holds samples
    # 4p..4p+3 back to back (x_shard.reshape(128, 2048) — contiguous).
    x_d = nc.dram_tensor(
        "x", [P, N_CHUNKS * FEAT], mybir.dt.float32, kind="ExternalInput"
    )
    lab_d = nc.dram_tensor(
        "labels", [P, N_CHUNKS], mybir.dt.int32, kind="ExternalInput"
    )
    cen_d = nc.dram_tensor(
        "centers", [NUM_CLASSES, FEAT], mybir.dt.float32, kind="ExternalInput"
    )
    # out[p, k] = squared distance of sample 4p + k.
    out_d = nc.dram_tensor(
        "out", [P, N_CHUNKS], mybir.dt.float32, kind="ExternalOutput"
    )

    lab_t = nc.alloc_sbuf_tensor("lab_t", [P, N_CHUNKS], mybir.dt.int32)
    x_t = nc.alloc_sbuf_tensor("x_t", [P, N_CHUNKS * FEAT], mybir.dt.float32)
    c_t = [
        nc.alloc_sbuf_tensor(f"c_t{k}", [P, FEAT], mybir.dt.float32)
        for k in range(N_CHUNKS)
    ]
    d_t = [
        nc.alloc_sbuf_tensor(f"d_t{k}", [P, FEAT], mybir.dt.float32)
        for k in range(N_CHUNKS)
    ]
    sq_t = [
        nc.alloc_sbuf_tensor(f"sq_t{k}", [P, FEAT], mybir.dt.float32)
        for k in range(N_CHUNKS)
    ]
    res_t = nc.alloc_sbuf_tensor("res_t", [P, N_CHUNKS], mybir.dt.float32)

    sem_lab = nc.alloc_semaphore("sem_lab")
    sem_x = nc.alloc_semaphore("sem_x")
    sem_g = [nc.alloc_semaphore(f"sem_g{k}") for k in range(N_CHUNKS)]
    sem_v = nc.alloc_semaphore("sem_v")
    sem_vt = nc.alloc_semaphore("sem_vt")
    sem_a = nc.alloc_semaphore("sem_a")
    sem_out = nc.alloc_semaphore("sem_out")

    with nc.Block() as block:

        @block.sync
        def _(sync):
            sync.dma_start(out=lab_t[:], in_=lab_d[:, :]).then_inc(sem_lab, 16)
            # Out DMA: wait for ACT's three accum columns + DVE's last one.
            sync.wait_ge(sem_a, N_CHUNKS - 1)
            sync.wait_ge(sem_vt, 1)
            # No explicit completion wait: sem_out is still attached so the
            # Bacc epilogue drain quiesces the DMA before the NEFF ends,
            # without stalling SP on the ~900ns completion-sem round trip.
            sync.dma_start(out=out_d[:, :], in_=res_t[:]).then_inc(sem_out, 16)

        @block.gpsimd
        def _(gpsimd):
            # x via Pool SWDGE: descriptor-gen runs on the otherwise-idle
            # GpSimd engine right after the entry barrier, so the x transfer
            # hits the DMA bus earlier than an HWDGE issue queued behind the
            # labels DMA.
            gpsimd.dma_start(out=x_t[:], in_=x_d[:, :]).then_inc(sem_x, 16)
            gpsimd.wait_ge(sem_lab, 16)
            for k in range(N_CHUNKS):
                gi = gpsimd.indirect_dma_start(
                    out=c_t[k][:],
                    out_offset=None,
                    in_=cen_d[:],
                    in_offset=bass.IndirectOffsetOnAxis(
                        ap=lab_t[:, k : k + 1], axis=0
                    ),
                )
                # Alternate SWDGE queues: descriptor-gen for consecutive
                # gathers can run on parallel Q7 queues on HW (the serial
                # 4x~1us gen chain is the critical-path pacer; the cost
                # model serializes it either way). Out-of-order completion
                # across queues is already handled by per-gather sems.
                if k % 2 == 1:
                    gi.ins.queue = "qPoolDynamic1"
                gi.then_inc(sem_g[k], 16)

        @block.vector
        def _(vector):
            vector.wait_ge(sem_x, 16)
            for k in range(N_CHUNKS):
                vector.wait_ge(sem_g[k], 16)
                vector.tensor_tensor(
                    out=d_t[k][:],
                    in0=x_t[:, k * FEAT : (k + 1) * FEAT],
                    in1=c_t[k][:],
                    op=mybir.AluOpType.subtract,
                ).then_inc(sem_v, 1)
            # DVE is deep-pipelined: the reduce must wait its own engine's
            # subtract retire before reading d_t3.
            vector.wait_ge(sem_v, N_CHUNKS)
            # Square + free-axis accumulate in one standard TensorScalarPtr:
            # out = (d + 0) * d, accum = sum(out). (tensor_tensor_reduce is a
            # custom DVE op that faults through this execution path.)
            vector.scalar_tensor_tensor(
                out=sq_t[N_CHUNKS - 1][:],
                in0=d_t[N_CHUNKS - 1][:],
                scalar=0.0,
                in1=d_t[N_CHUNKS - 1][:],
                op0=mybir.AluOpType.add,
                op1=mybir.AluOpType.mult,
                accum_out=res_t[:, N_CHUNKS - 1 : N_CHUNKS],
            ).then_inc(sem_vt, 1)

        @block.scalar
        def _(scalar):
            for k in range(N_CHUNKS - 1):
                scalar.wait_ge(sem_v, k + 1)
                scalar.activation(
                    out=sq_t[k][:],
                    in_=d_t[k][:],
                    func=mybir.ActivationFunctionType.Square,
                    accum_out=res_t[:, k : k + 1],
                ).then_inc(sem_a, 1)

    nc.compile()
    return nc


def kernel(x, centers, labels):
    global _cached_nc, LAST_RESULT
    if _cached_nc is None:
        _cached_nc = _build_nc()
    nc = _cached_nc

    x = np.ascontiguousarray(x, dtype=np.float32)
    centers = np.ascontiguousarray(centers, dtype=np.float32)
    labels_i32 = np.ascontiguousarray(labels.astype(np.int32))

    in_maps = []
    for c in range(N_CORES):
        sl = slice(c * SHARD, (c + 1) * SHARD)
        in_maps.append(
            {
                "x": x[sl].reshape(P, N_CHUNKS * FEAT),
                "labels": labels_i32[sl].reshape(P, N_CHUNKS),
                "centers": centers,
            }
        )

    try:
        LAST_RESULT = run_bass_kernel_spmd(nc, in_maps, core_ids=list(range(N_CORES)))
    except ModuleNotFoundError:
        # BASS_TRACE=1 under axon needs antenv.axon_hooks, which some
        # containers lack; fall back to an untraced run instead of crashing.
        os.environ["BASS_NEVER_TRACE"] = "1"
        LAST_RESULT = run_bass_kernel_spmd(nc, in_maps, core_ids=list(range(N_CORES)))

    # out[p, k] = distance of sample 4p + k -> natural order after reshape.
    dist = np.concatenate([r["out"].reshape(-1) for r in LAST_RESULT.results])
    dist = np.clip(dist, CLAMP_MIN, CLAMP_MAX)
    return np.asarray(dist.mean(dtype=np.float64), dtype=np.float32)



# revision 3
# speedup vs baseline: 1.2316x; 1.2316x over previous
"""CenterLoss kernel for Trainium2 (8 NeuronCores, SPMD data-parallel).

Math (per reference):
    c_i   = centers[labels[i]]                  # gather, (B, D)
    d_i   = ||x_i - c_i||^2
    out   = mean(clip(d_i, 1e-12, 1e12))

Strategy (target_regime=memory):
  - Shard the batch (4096) across 8 cores -> 512 samples/core; partition p
    handles samples 4p..4p+3.
  - x upload is a *static* copy: one Pool SWDGE DMA with f32->bf16 cast in
    the datapath (halves SBUF-side bytes; bf16 keeps the mean's rel-err at
    ~1e-4, far under the 2e-2 gate). Its descriptor gen starts immediately.
  - The center rows go through two indirect row-gathers (2 samples each)
    from the full replicated centers table, also casting f32->bf16. Their
    offsets (labels) must sit in SBUF (walrus: "Vector-dynamic-offsets
    location must be SB"), so a small HWDGE DMA stages gidx first; the two
    gathers pipeline compute under the second transfer.
  - DVE subtracts x-c per sample (bf16 2x mode); squares+accums are split
    between ACT (Square activation w/ accum) and DVE (scalar_tensor_tensor
    w/ accum) into res[:, 0:4] (f32).
  - Output leaves via a *prepared* dma_scatter_add (descriptor gen runs
    early, off the critical path) fired by trigger_dma after the last
    accum -- the tail is transfer+sem instead of a full HWDGE chain.
    Scatter indices are an identity permutation; PJRT/native runners
    pre-zero ExternalOutput buffers so '+=' lands the plain values.
  - Each core returns 512 per-sample distances in out[:, 0:4]; the host
    does the final clip + mean (the unshard step).
"""

import os

import numpy as np

import concourse.bacc as bacc
import concourse.bass as bass
import concourse.mybir as mybir
from concourse.bass_utils import run_bass_kernel_spmd

N_CORES = 8
BATCH = 4096
FEAT = 512
NUM_CLASSES = 50000
SHARD = BATCH // N_CORES  # 512 samples per core
P = 128
SPP = SHARD // P  # 4 samples per partition
HALF = SPP // 2  # samples per gather chunk (2)

CLAMP_MIN = 1e-12
CLAMP_MAX = 1e12

OUT_COLS = 64  # scatter-add elem_size floor: 64 f32 = 256B

_cached_nc = None

# Last BassKernelResults (for test harnesses that want exec_time_ns).
LAST_RESULT = None


def _build_nc():
    nc = bacc.Bacc("TRN2", target_bir_lowering=False, debug=False, num_swdge_queues=2)

    cen_d = nc.dram_tensor(
        "centers", [NUM_CLASSES, FEAT], mybir.dt.float32, kind="ExternalInput"
    )
    # x pre-reshaped host-side to [128, 4*512]: partition p holds samples
    # 4p..4p+3 back to back (contiguous slice of the shard).
    x_d = nc.dram_tensor("x", [P, SPP * FEAT], mybir.dt.float32, kind="ExternalInput")
    # Gather row indices, partition p: [y(4p), y(4p+1), y(4p+2), y(4p+3)].
    gidx_d = nc.dram_tensor("gidx", [P, SPP], mybir.dt.int32, kind="ExternalInput")
    # Scatter identity indices, wrapped in 16 partitions (row-replicated).
    sidx_d = nc.dram_tensor("sidx", [P, 8], mybir.dt.int16, kind="ExternalInput")
    # out[p, k] = squared distance of sample 4p + k (cols 4.. unused pad).
    out_d = nc.dram_tensor(
        "out", [P, OUT_COLS], mybir.dt.float32, kind="ExternalOutput"
    )

    x_t = nc.alloc_sbuf_tensor("x_t", [P, SPP * FEAT], mybir.dt.bfloat16)
    c_t = [
        nc.alloc_sbuf_tensor(f"c_t{g}", [P, HALF * FEAT], mybir.dt.bfloat16)
        for g in range(2)
    ]
    d_t = [
        nc.alloc_sbuf_tensor(f"d{g}", [P, HALF * FEAT], mybir.dt.bfloat16)
        for g in range(2)
    ]
    sq_t = nc.alloc_sbuf_tensor("sq", [P, SPP * FEAT], mybir.dt.bfloat16)
    res_t = nc.alloc_sbuf_tensor("res", [P, OUT_COLS], mybir.dt.float32)
    gidx_t = nc.alloc_sbuf_tensor("gidx_t", [P, SPP], mybir.dt.int32)
    sidx_t = nc.alloc_sbuf_tensor("sidx_t", [P, 8], mybir.dt.int16)

    s_x = nc.alloc_semaphore("s_x")
    s_g = [nc.alloc_semaphore(f"s_g{g}") for g in range(2)]
    s_gidx = nc.alloc_semaphore("s_gidx")
    s_sidx = nc.alloc_semaphore("s_sidx")
    s_sub = [nc.alloc_semaphore(f"s_sub{g}") for g in range(2)]
    s_qa = nc.alloc_semaphore("s_qa")  # ACT accums
    s_qd = nc.alloc_semaphore("s_qd")  # DVE accums
    s_prep = nc.alloc_semaphore("s_prep")
    s_out = nc.alloc_semaphore("s_out")

    with nc.Block() as block:

        @block.sync
        def _(sync):
            # gidx gates the gather desc-gen: absolutely first on HWDGE.
            sync.dma_start(out=gidx_t[:], in_=gidx_d[:, :]).then_inc(s_gidx, 16)
            # sidx is only needed by the scatter-prep (~4.5us in); queue it
            # behind gidx on the same ring.
            sync.dma_start(out=sidx_t[:], in_=sidx_d[:, :]).then_inc(s_sidx, 16)

        @block.gpsimd
        def _(gpsimd):
            # x cast-DMA: static, desc-gen starts immediately, transfer fills
            # the DMA pipe while gidx/gather-gen latency elapses.
            gpsimd.dma_start(out=x_t[:], in_=x_d[:, :]).then_inc(s_x, 16)

            # Two center-row gathers (2 samples each), f32->bf16 cast.
            gpsimd.wait_ge(s_gidx, 16)
            for g in range(2):
                gpsimd.indirect_dma_start(
                    out=c_t[g][:],
                    out_offset=None,
                    in_=cen_d[:],
                    in_offset=bass.IndirectOffsetOnAxis(
                        ap=gidx_t[:, g * HALF : (g + 1) * HALF], axis=0
                    ),
                ).then_inc(s_g[g], 16)

            # Prepare the result scatter now (desc-gen off the critical
            # path); fired by trigger_dma after the last accumulation.
            gpsimd.wait_ge(s_sidx, 16)
            gpsimd.dma_scatter_add(
                out_ap=out_d[:, :],
                in_ap=res_t[:].unsqueeze(1),
                idxs_ap=sidx_t[:],
                num_idxs=P,
                num_idxs_reg=P,
                elem_size=OUT_COLS,
                prepare_only=True,
                sem=s_out,
            ).then_inc(s_prep, 1)
            gpsimd.wait_ge(s_prep, 1)
            gpsimd.wait_ge(s_qa, 2)
            gpsimd.wait_ge(s_qd, 2)
            gpsimd.trigger_dma(count=1)
            gpsimd.wait_ge(s_out, 16)

        @block.vector
        def _(vector):
            # res cols 4.. are scattered as pad; keep them defined.
            vector.memset(res_t[:], 0.0)
            vector.wait_ge(s_x, 16)
            for g in range(2):
                vector.wait_ge(s_g[g], 16)
                for h in range(HALF):
                    k = g * HALF + h
                    # Per-sample subtract so ACT can square sample 2g while
                    # DVE moves on.
                    vector.tensor_tensor(
                        out=d_t[g][:, h * FEAT : (h + 1) * FEAT],
                        in0=x_t[:, k * FEAT : (k + 1) * FEAT],
                        in1=c_t[g][:, h * FEAT : (h + 1) * FEAT],
                        op=mybir.AluOpType.subtract,
                    ).then_inc(s_sub[g], 1)
                # DVE squares+accums the pair's second sample (ACT takes the
                # first). Same-engine program order covers the d_t read.
                k = g * HALF + 1
                vector.wait_ge(s_sub[g], 2)
                vector.scalar_tensor_tensor(
                    out=sq_t[:, k * FEAT : (k + 1) * FEAT],
                    in0=d_t[g][:, FEAT : 2 * FEAT],
                    scalar=0.0,
                    in1=d_t[g][:, FEAT : 2 * FEAT],
                    op0=mybir.AluOpType.add,
                    op1=mybir.AluOpType.mult,
                    accum_out=res_t[:, k : k + 1],
                ).then_inc(s_qd, 1)

        @block.scalar
        def _(scalar):
            for g in range(2):
                k = g * HALF
                scalar.wait_ge(s_sub[g], 1)
                scalar.activation(
                    out=sq_t[:, k * FEAT : (k + 1) * FEAT],
                    in_=d_t[g][:, 0:FEAT],
                    func=mybir.ActivationFunctionType.Square,
                    accum_out=res_t[:, k : k + 1],
                ).then_inc(s_qa, 1)

    nc.compile()
    return nc


def _host_inputs(x, centers, labels_i32):
    """Per-core input dicts."""
    # Scatter identity: unwrapped index k lives at [k % 16, k // 16];
    # replicate the 16-partition block across all 128 partitions.
    sidx16 = (
        np.arange(8, dtype=np.int16)[None, :] * 16
        + np.arange(16, dtype=np.int16)[:, None]
    )  # [16, 8], sidx16[ch, s] = s*16 + ch
    sidx = np.ascontiguousarray(np.tile(sidx16, (8, 1)))  # [128, 8]

    in_maps = []
    for c in range(N_CORES):
        sl = slice(c * SHARD, (c + 1) * SHARD)
        in_maps.append(
            {
                "centers": centers,
                "x": x[sl].reshape(P, SPP * FEAT),
                "gidx": np.ascontiguousarray(labels_i32[sl].reshape(P, SPP)),
                "sidx": sidx,
            }
        )
    return in_maps


def kernel(x, centers, labels):
    global _cached_nc, LAST_RESULT
    if _cached_nc is None:
        _cached_nc = _build_nc()
    nc = _cached_nc

    x = np.ascontiguousarray(x, dtype=np.float32)
    centers = np.ascontiguousarray(centers, dtype=np.float32)
    labels_i32 = np.ascontiguousarray(labels.astype(np.int32))

    in_maps = _host_inputs(x, centers, labels_i32)

    try:
        LAST_RESULT = run_bass_kernel_spmd(nc, in_maps, core_ids=list(range(N_CORES)))
    except ModuleNotFoundError:
        # BASS_TRACE=1 under axon needs antenv.axon_hooks, which some
        # containers lack; fall back to an untraced run instead of crashing.
        os.environ["BASS_NEVER_TRACE"] = "1"
        LAST_RESULT = run_bass_kernel_spmd(nc, in_maps, core_ids=list(range(N_CORES)))

    # out[p, 0:4] = distances of samples 4p+0..3 -> natural order.
    dist = np.concatenate(
        [r["out"][:, 0:SPP].reshape(-1) for r in LAST_RESULT.results]
    )
    dist = np.clip(dist, CLAMP_MIN, CLAMP_MAX)
    return np.asarray(dist.mean(dtype=np.float64), dtype=np.float32)


# revision 6
# speedup vs baseline: 1.3128x; 1.0659x over previous
"""CenterLoss kernel for Trainium2 (8 NeuronCores, SPMD data-parallel).

Math (per reference):
    c_i   = centers[labels[i]]                  # gather, (B, D)
    d_i   = ||x_i - c_i||^2
    out   = mean(clip(d_i, 1e-12, 1e12))

Strategy (target_regime=memory):
  - Shard the batch (4096) across 8 cores -> 512 samples/core; partition p
    handles samples 4p..4p+3.
  - x upload is a *static* copy: one Pool SWDGE DMA with f32->bf16 cast in
    the datapath (halves SBUF-side bytes; bf16 keeps the mean's rel-err at
    ~1e-4, far under the 2e-2 gate). Its descriptor gen starts immediately.
  - The center rows go through two indirect row-gathers (2 samples each)
    from the full replicated centers table, also casting f32->bf16. Their
    offsets (labels) must sit in SBUF (walrus: "Vector-dynamic-offsets
    location must be SB"), so a small HWDGE DMA stages gidx first; the two
    gathers pipeline compute under the second transfer.
  - DVE subtracts x-c per sample (bf16 2x mode); squares+accums are split
    between ACT (Square activation w/ accum) and DVE (scalar_tensor_tensor
    w/ accum) into res[:, 0:4] (f32).
  - Output leaves via a *prepared* dma_scatter_add (descriptor gen runs
    early, off the critical path) fired by trigger_dma after the last
    accum -- the tail is transfer+sem instead of a full HWDGE chain.
    Scatter indices are an identity permutation; PJRT/native runners
    pre-zero ExternalOutput buffers so '+=' lands the plain values.
  - Each core returns 512 per-sample distances in out[:, 0:4]; the host
    does the final clip + mean (the unshard step).
"""

import os

import numpy as np

import concourse.bacc as bacc
import concourse.bass as bass
import concourse.mybir as mybir
from concourse.bass_utils import run_bass_kernel_spmd

N_CORES = 8
BATCH = 4096
FEAT = 512
NUM_CLASSES = 50000
SHARD = BATCH // N_CORES  # 512 samples per core
P = 128
SPP = SHARD // P  # 4 samples per partition
HALF = SPP // 2  # samples per gather chunk (2)

CLAMP_MIN = 1e-12
CLAMP_MAX = 1e12

OUT_COLS = 64  # scatter-add elem_size floor: 64 f32 = 256B

_cached_nc = None

# Last BassKernelResults (for test harnesses that want exec_time_ns).
LAST_RESULT = None


def _build_nc():
    nc = bacc.Bacc("TRN2", target_bir_lowering=False, debug=False, num_swdge_queues=2)

    cen_d = nc.dram_tensor(
        "centers", [NUM_CLASSES, FEAT], mybir.dt.float32, kind="ExternalInput"
    )
    # x pre-reshaped host-side to [128, 4*512]: partition p holds samples
    # 4p..4p+3 back to back (contiguous slice of the shard).
    x_d = nc.dram_tensor("x", [P, SPP * FEAT], mybir.dt.float32, kind="ExternalInput")
    # Gather row indices, partition p: [y(4p), y(4p+1), y(4p+2), y(4p+3)].
    gidx_d = nc.dram_tensor("gidx", [P, SPP], mybir.dt.int32, kind="ExternalInput")
    # Scatter identity indices, wrapped in 16 partitions (row-replicated).
    sidx_d = nc.dram_tensor("sidx", [P, 8], mybir.dt.int16, kind="ExternalInput")
    # out[p, k] = squared distance of sample 4p + k (cols 4.. unused pad).
    out_d = nc.dram_tensor(
        "out", [P, OUT_COLS], mybir.dt.float32, kind="ExternalOutput"
    )

    x_t = nc.alloc_sbuf_tensor("x_t", [P, SPP * FEAT], mybir.dt.bfloat16)
    c_t = [
        nc.alloc_sbuf_tensor(f"c_t{g}", [P, HALF * FEAT], mybir.dt.bfloat16)
        for g in range(2)
    ]
    d_t = [
        nc.alloc_sbuf_tensor(f"d{g}", [P, HALF * FEAT], mybir.dt.bfloat16)
        for g in range(2)
    ]
    sq_t = nc.alloc_sbuf_tensor("sq", [P, SPP * FEAT], mybir.dt.bfloat16)
    res_t = nc.alloc_sbuf_tensor("res", [P, OUT_COLS], mybir.dt.float32)
    gidx_t = nc.alloc_sbuf_tensor("gidx_t", [P, SPP], mybir.dt.int32)
    sidx_t = nc.alloc_sbuf_tensor("sidx_t", [P, 8], mybir.dt.int16)

    s_x = nc.alloc_semaphore("s_x")
    s_g = [nc.alloc_semaphore(f"s_g{g}") for g in range(2)]
    s_gidx = nc.alloc_semaphore("s_gidx")
    s_sidx = nc.alloc_semaphore("s_sidx")
    s_sub = [nc.alloc_semaphore(f"s_sub{g}") for g in range(2)]
    s_qa = nc.alloc_semaphore("s_qa")  # ACT accums
    s_qd = nc.alloc_semaphore("s_qd")  # DVE accums
    s_prep = nc.alloc_semaphore("s_prep")
    s_out = nc.alloc_semaphore("s_out")

    with nc.Block() as block:

        @block.sync
        def _(sync):
            # gidx gates the gather desc-gen: absolutely first on HWDGE.
            sync.dma_start(out=gidx_t[:], in_=gidx_d[:, :]).then_inc(s_gidx, 16)
            # sidx is only needed by the scatter-prep (~4.5us in); queue it
            # behind gidx on the same ring.
            sync.dma_start(out=sidx_t[:], in_=sidx_d[:, :]).then_inc(s_sidx, 16)

        @block.gpsimd
        def _(gpsimd):
            # x cast-DMA: static, desc-gen starts immediately, transfer fills
            # the DMA pipe while gidx/gather-gen latency elapses.
            gpsimd.dma_start(out=x_t[:], in_=x_d[:, :]).then_inc(s_x, 16)

            # Two center-row gathers (2 samples each), f32->bf16 cast.
            gpsimd.wait_ge(s_gidx, 16)
            for g in range(2):
                gpsimd.indirect_dma_start(
                    out=c_t[g][:],
                    out_offset=None,
                    in_=cen_d[:],
                    in_offset=bass.IndirectOffsetOnAxis(
                        ap=gidx_t[:, g * HALF : (g + 1) * HALF], axis=0
                    ),
                ).then_inc(s_g[g], 16)

            # Prepare the result scatter now (desc-gen off the critical
            # path); fired by trigger_dma after the last accumulation.
            gpsimd.wait_ge(s_sidx, 16)
            gpsimd.dma_scatter_add(
                out_ap=out_d[:, :],
                in_ap=res_t[:].unsqueeze(1),
                idxs_ap=sidx_t[:],
                num_idxs=P,
                num_idxs_reg=P,
                elem_size=OUT_COLS,
                prepare_only=True,
                sem=s_out,
            ).then_inc(s_prep, 1)
            gpsimd.wait_ge(s_prep, 1)
            gpsimd.wait_ge(s_qa, 2)
            gpsimd.wait_ge(s_qd, 2)
            gpsimd.trigger_dma(count=1)
            # No explicit s_out wait: the sem is baked into the scatter
            # descriptor, so the Bacc epilogue dge-drain quiesces the DMA
            # before the NEFF ends without stalling Pool on the ~900ns
            # completion round trip (same idiom the stock HWDGE-out used).

        @block.vector
        def _(vector):
            # res cols 4.. are scattered as pad; keep them defined.
            vector.memset(res_t[:], 0.0)
            vector.wait_ge(s_x, 16)
            for g in range(2):
                vector.wait_ge(s_g[g], 16)
                if g == 0:
                    # Pair 1: one merged subtract (saves an op's fixed cost
                    # so stp_s1 retires before pair 2's gather sem fires).
                    vector.tensor_tensor(
                        out=d_t[g][:],
                        in0=x_t[:, 0 : HALF * FEAT],
                        in1=c_t[g][:],
                        op=mybir.AluOpType.subtract,
                    ).then_inc(s_sub[g], 2)
                else:
                    # Pair 2: per-sample subtract so ACT can start squaring
                    # sample 2 while DVE subtracts sample 3.
                    for h in range(HALF):
                        k = g * HALF + h
                        vector.tensor_tensor(
                            out=d_t[g][:, h * FEAT : (h + 1) * FEAT],
                            in0=x_t[:, k * FEAT : (k + 1) * FEAT],
                            in1=c_t[g][:, h * FEAT : (h + 1) * FEAT],
                            op=mybir.AluOpType.subtract,
                        ).then_inc(s_sub[g], 1)
                # DVE squares+accums the pair's second sample (ACT takes the
                # first). Same-engine program order covers the d_t read.
                k = g * HALF + 1
                vector.wait_ge(s_sub[g], 2)
                vector.scalar_tensor_tensor(
                    out=sq_t[:, k * FEAT : (k + 1) * FEAT],
                    in0=d_t[g][:, FEAT : 2 * FEAT],
                    scalar=0.0,
                    in1=d_t[g][:, FEAT : 2 * FEAT],
                    op0=mybir.AluOpType.add,
                    op1=mybir.AluOpType.mult,
                    accum_out=res_t[:, k : k + 1],
                ).then_inc(s_qd, 1)

        @block.scalar
        def _(scalar):
            for g in range(2):
                k = g * HALF
                scalar.wait_ge(s_sub[g], 1)
                scalar.activation(
                    out=sq_t[:, k * FEAT : (k + 1) * FEAT],
                    in_=d_t[g][:, 0:FEAT],
                    func=mybir.ActivationFunctionType.Square,
                    # Bias from a zeroed res pad column instead of the
                    # const-0.0 AP: the s_sub wait transitively orders DVE's
                    # memset before this read, and it makes the 4 const-AP
                    # preamble memsets dead so they can be stripped below.
                    bias=res_t[:, SPP : SPP + 1],
                    accum_out=res_t[:, k : k + 1],
                ).then_inc(s_qa, 1)

    # Strip the (now-unreferenced) const-AP registration memsets from the
    # preamble: they serialize on Pool ahead of the entry barrier and delay
    # every engine's first instruction by ~0.5us.
    const_names = {"const-float32-0.0", "const-float32-1.0",
                   "const-bfloat16-1.0", "const-uint8-127"}
    for blk in nc.m.functions[0].blocks:
        keep = []
        for ins in blk.instructions:
            if type(ins).__name__ == "InstMemset":
                outs = ins.outs
                tname = None
                try:
                    tname = outs[0].bass_ap.tensor.name
                except Exception:
                    pass
                if tname in const_names:
                    continue
            keep.append(ins)
        if len(keep) != len(blk.instructions):
            blk.instructions[:] = keep

    nc.compile()
    return nc


def _host_inputs(x, centers, labels_i32):
    """Per-core input dicts."""
    # Scatter identity: unwrapped index k lives at [k % 16, k // 16];
    # replicate the 16-partition block across all 128 partitions.
    sidx16 = (
        np.arange(8, dtype=np.int16)[None, :] * 16
        + np.arange(16, dtype=np.int16)[:, None]
    )  # [16, 8], sidx16[ch, s] = s*16 + ch
    sidx = np.ascontiguousarray(np.tile(sidx16, (8, 1)))  # [128, 8]

    in_maps = []
    for c in range(N_CORES):
        sl = slice(c * SHARD, (c + 1) * SHARD)
        in_maps.append(
            {
                "centers": centers,
                "x": x[sl].reshape(P, SPP * FEAT),
                "gidx": np.ascontiguousarray(labels_i32[sl].reshape(P, SPP)),
                "sidx": sidx,
            }
        )
    return in_maps


def kernel(x, centers, labels):
    global _cached_nc, LAST_RESULT
    if _cached_nc is None:
        _cached_nc = _build_nc()
    nc = _cached_nc

    x = np.ascontiguousarray(x, dtype=np.float32)
    centers = np.ascontiguousarray(centers, dtype=np.float32)
    labels_i32 = np.ascontiguousarray(labels.astype(np.int32))

    in_maps = _host_inputs(x, centers, labels_i32)

    try:
        LAST_RESULT = run_bass_kernel_spmd(nc, in_maps, core_ids=list(range(N_CORES)))
    except ModuleNotFoundError:
        # BASS_TRACE=1 under axon needs antenv.axon_hooks, which some
        # containers lack; fall back to an untraced run instead of crashing.
        os.environ["BASS_NEVER_TRACE"] = "1"
        LAST_RESULT = run_bass_kernel_spmd(nc, in_maps, core_ids=list(range(N_CORES)))

    # out[p, 0:4] = distances of samples 4p+0..3 -> natural order.
    dist = np.concatenate(
        [r["out"][:, 0:SPP].reshape(-1) for r in LAST_RESULT.results]
    )
    dist = np.clip(dist, CLAMP_MIN, CLAMP_MAX)
    return np.asarray(dist.mean(dtype=np.float64), dtype=np.float32)


# revision 13
# speedup vs baseline: 1.3742x; 1.0468x over previous
"""CenterLoss kernel for Trainium2 (8 NeuronCores, SPMD data-parallel).

Math (per reference):
    c_i   = centers[labels[i]]                  # gather, (B, D)
    d_i   = ||x_i - c_i||^2
    out   = mean(clip(d_i, 1e-12, 1e12))

Strategy (target_regime=memory):
  - Shard the batch (4096) across 8 cores -> 512 samples/core; partition p
    handles samples 4p..4p+3.
  - x upload is a static copy: one Pool SWDGE DMA with f32->bf16 cast in
    the datapath (halves SBUF-side bytes; bf16 keeps the mean's rel-err at
    ~1e-4, far under the 2e-2 gate). Its descriptor gen starts immediately
    and its transfer hides under the gather-index latency.
  - The center rows come via two indirect row-gathers (2 samples each)
    from the replicated table, also casting f32->bf16, pipelined so the
    first pair's compute overlaps the second pair's transfer. Gather
    offsets (labels) must sit in SBUF (walrus: "Vector-dynamic-offsets
    location must be SB"), so a small HWDGE DMA stages gidx first.
    (A fancier variant that used the DMA compute-op to fuse the subtract
    was abandoned: with >1 index per partition the real ucode garbles all
    but the first row, though it passes the Python interpreter.)
  - DVE subtracts x-c per sample (bf16 2x mode). Squares+accums are split
    across ACT (Square activation w/ accum) and DVE (scalar_tensor_tensor
    w/ accum): ACT takes s0, the 192-col tail of s1, and s2; DVE takes the
    320-col head of s1 and s3. The split sizes DVE's pair-1 work to fit
    inside the gather-1->gather-2 gap so pair-2's subtracts start the
    moment gather 2 lands. s1's two partial accums land in separate res
    columns; the host adds them.
  - Output leaves via a *prepared* dma_scatter_add (descriptor gen early,
    off the critical path) fired by trigger_dma after the last accum; the
    tail is transfer+sem instead of a full HWDGE chain. Scatter indices
    are an identity permutation; PJRT/native runners pre-zero
    ExternalOutput buffers so '+=' lands the plain values. The completion
    sem is baked into the descriptor and quiesced by the epilogue
    dge-drain.
  - The stock preamble (4 const-AP memsets + entry all-engine barrier) is
    stripped: the activation bias comes from a zeroed res pad column, so
    nothing needs the barrier. Barrier sems self-reset, so the exit
    barrier is unaffected. A dummy 1-element Square anchors the ACT table
    load into idle time instead of the critical path.
  - Each core returns per-sample distances in res columns {0,1+5,2,3}; the
    host does the final clip + mean (the unshard step).
"""

import os

import numpy as np

import concourse.bacc as bacc
import concourse.bass as bass
import concourse.mybir as mybir
from concourse.bass_utils import run_bass_kernel_spmd

N_CORES = 8
BATCH = 4096
FEAT = 512
NUM_CLASSES = 50000
SHARD = BATCH // N_CORES  # 512 samples per core
P = 128
SPP = SHARD // P  # 4 samples per partition
HALF = SPP // 2  # samples per gather chunk (2)

CLAMP_MIN = 1e-12
CLAMP_MAX = 1e12

OUT_COLS = 64  # scatter-add elem_size floor: 64 f32 = 256B
S1_SPLIT = 320  # cols of sample 1's square on DVE; rest on ACT
BIAS_COL = 4  # zeroed res column used as activation bias
S1B_COL = 5  # ACT's partial accum for sample 1 (host adds to col 1)

_cached_nc = None

# Last BassKernelResults (for test harnesses that want exec_time_ns).
LAST_RESULT = None


def _build_nc():
    nc = bacc.Bacc("TRN2", target_bir_lowering=False, debug=False, num_swdge_queues=2)

    cen_d = nc.dram_tensor(
        "centers", [NUM_CLASSES, FEAT], mybir.dt.float32, kind="ExternalInput"
    )
    # x pre-reshaped host-side to [128, 4*512]: partition p holds samples
    # 4p..4p+3 back to back.
    x_d = nc.dram_tensor("x", [P, SPP * FEAT], mybir.dt.float32, kind="ExternalInput")
    # Gather row indices, partition p: [y(4p), y(4p+1), y(4p+2), y(4p+3)].
    gidx_d = nc.dram_tensor("gidx", [P, SPP], mybir.dt.int32, kind="ExternalInput")
    # Scatter identity indices, wrapped in 16 partitions (row-replicated).
    sidx_d = nc.dram_tensor("sidx", [P, 8], mybir.dt.int16, kind="ExternalInput")
    # out[p, k] = squared-distance accums of samples 4p+k (see res layout).
    out_d = nc.dram_tensor(
        "out", [P, OUT_COLS], mybir.dt.float32, kind="ExternalOutput"
    )

    x_t = nc.alloc_sbuf_tensor("x_t", [P, SPP * FEAT], mybir.dt.bfloat16)
    c_t = [
        nc.alloc_sbuf_tensor(f"c_t{g}", [P, HALF * FEAT], mybir.dt.bfloat16)
        for g in range(2)
    ]
    d_t = [
        nc.alloc_sbuf_tensor(f"d{g}", [P, HALF * FEAT], mybir.dt.bfloat16)
        for g in range(2)
    ]
    sq_t = nc.alloc_sbuf_tensor("sq", [P, SPP * FEAT], mybir.dt.bfloat16)
    res_t = nc.alloc_sbuf_tensor("res", [P, OUT_COLS], mybir.dt.float32)
    gidx_t = nc.alloc_sbuf_tensor("gidx_t", [P, SPP], mybir.dt.int32)
    sidx_t = nc.alloc_sbuf_tensor("sidx_t", [P, 8], mybir.dt.int16)

    s_x = nc.alloc_semaphore("s_x")
    s_g = [nc.alloc_semaphore(f"s_g{g}") for g in range(2)]
    s_gidx = nc.alloc_semaphore("s_gidx")
    s_sidx = nc.alloc_semaphore("s_sidx")
    s_z = nc.alloc_semaphore("s_z")  # res memset done (ACT bias read)
    s_sub = [nc.alloc_semaphore(f"s_sub{g}") for g in range(2)]
    s_qa = nc.alloc_semaphore("s_qa")  # ACT accums
    s_qd = nc.alloc_semaphore("s_qd")  # DVE accums
    s_prep = nc.alloc_semaphore("s_prep")
    s_out = nc.alloc_semaphore("s_out")

    with nc.Block() as block:

        @block.sync
        def _(sync):
            # gidx gates the gather desc-gen: absolutely first on HWDGE.
            sync.dma_start(out=gidx_t[:], in_=gidx_d[:, :]).then_inc(s_gidx, 16)
            # sidx is only needed by the scatter-prep (~4.5us in).
            sync.dma_start(out=sidx_t[:], in_=sidx_d[:, :]).then_inc(s_sidx, 16)

        @block.gpsimd
        def _(gpsimd):
            # x cast-DMA: static, desc-gen starts immediately; the transfer
            # fills the DMA pipe while the gidx chain and gather desc-gen
            # latency elapse.
            gpsimd.dma_start(out=x_t[:], in_=x_d[:, :]).then_inc(s_x, 16)

            # Two center-row gathers (2 samples each), f32->bf16 cast.
            gpsimd.wait_ge(s_gidx, 16)
            for g in range(2):
                gpsimd.indirect_dma_start(
                    out=c_t[g][:],
                    out_offset=None,
                    in_=cen_d[:],
                    in_offset=bass.IndirectOffsetOnAxis(
                        ap=gidx_t[:, g * HALF : (g + 1) * HALF], axis=0
                    ),
                ).then_inc(s_g[g], 16)

            # Prepare the result scatter now (desc-gen off the critical
            # path); fired by trigger_dma after the last accumulation.
            gpsimd.wait_ge(s_sidx, 16)
            gpsimd.dma_scatter_add(
                out_ap=out_d[:, :],
                in_ap=res_t[:].unsqueeze(1),
                idxs_ap=sidx_t[:],
                num_idxs=P,
                num_idxs_reg=P,
                elem_size=OUT_COLS,
                prepare_only=True,
                sem=s_out,
            ).then_inc(s_prep, 1)
            gpsimd.wait_ge(s_prep, 1)
            gpsimd.wait_ge(s_qa, 3)
            gpsimd.wait_ge(s_qd, 2)
            gpsimd.trigger_dma(count=1)
            # No explicit s_out wait: the sem is baked into the scatter
            # descriptor, so the Bacc epilogue dge-drain quiesces the DMA
            # before the NEFF ends without stalling Pool on the ~900ns
            # completion round trip.

        @block.vector
        def _(vector):
            # res cols >= 4 are scattered as pad (col 4 doubles as the
            # activation bias); keep them defined.
            vector.memset(res_t[:], 0.0).then_inc(s_z, 1)
            vector.wait_ge(s_x, 16)
            for g in range(2):
                vector.wait_ge(s_g[g], 16)
                # Per-sample subtract so ACT can square each pair's first
                # sample while DVE moves on.
                for h in range(HALF):
                    k = g * HALF + h
                    vector.tensor_tensor(
                        out=d_t[g][:, h * FEAT : (h + 1) * FEAT],
                        in0=x_t[:, k * FEAT : (k + 1) * FEAT],
                        in1=c_t[g][:, h * FEAT : (h + 1) * FEAT],
                        op=mybir.AluOpType.subtract,
                    ).then_inc(s_sub[g], 1)
                if g == 0:
                    # Head of sample 1's square: sized so DVE is free again
                    # before gather 2's sem fires.
                    vector.wait_ge(s_sub[0], 2)
                    vector.scalar_tensor_tensor(
                        out=sq_t[:, FEAT : FEAT + S1_SPLIT],
                        in0=d_t[0][:, FEAT : FEAT + S1_SPLIT],
                        scalar=0.0,
                        in1=d_t[0][:, FEAT : FEAT + S1_SPLIT],
                        op0=mybir.AluOpType.add,
                        op1=mybir.AluOpType.mult,
                        accum_out=res_t[:, 1:2],
                    ).then_inc(s_qd, 1)
            # Sample 3's square+accum.
            vector.wait_ge(s_sub[1], 2)
            vector.scalar_tensor_tensor(
                out=sq_t[:, 3 * FEAT : 4 * FEAT],
                in0=d_t[1][:, FEAT : 2 * FEAT],
                scalar=0.0,
                in1=d_t[1][:, FEAT : 2 * FEAT],
                op0=mybir.AluOpType.add,
                op1=mybir.AluOpType.mult,
                accum_out=res_t[:, 3:4],
            ).then_inc(s_qd, 1)

        @block.scalar
        def _(scalar):
            # Order DVE's res memset before the bias-column reads below.
            scalar.wait_ge(s_z, 1)
            # Dummy 1-element Square: anchors the compiler-inserted
            # ACT_TABLE_LOAD here (~t=0.4us, idle time) instead of letting
            # it land behind the gather wait on the critical path.
            scalar.activation(
                out=sq_t[:, 0:1],
                in_=res_t[:, BIAS_COL + 1 : BIAS_COL + 2],
                func=mybir.ActivationFunctionType.Square,
                bias=res_t[:, BIAS_COL : BIAS_COL + 1],
            )
            # Sample 0.
            scalar.wait_ge(s_sub[0], 1)
            scalar.activation(
                out=sq_t[:, 0:FEAT],
                in_=d_t[0][:, 0:FEAT],
                func=mybir.ActivationFunctionType.Square,
                bias=res_t[:, BIAS_COL : BIAS_COL + 1],
                accum_out=res_t[:, 0:1],
            ).then_inc(s_qa, 1)
            # Tail of sample 1 (host adds this partial to res col 1).
            scalar.wait_ge(s_sub[0], 2)
            scalar.activation(
                out=sq_t[:, FEAT + S1_SPLIT : 2 * FEAT],
                in_=d_t[0][:, FEAT + S1_SPLIT : 2 * FEAT],
                func=mybir.ActivationFunctionType.Square,
                bias=res_t[:, BIAS_COL : BIAS_COL + 1],
                accum_out=res_t[:, S1B_COL : S1B_COL + 1],
            ).then_inc(s_qa, 1)
            # Sample 2.
            scalar.wait_ge(s_sub[1], 1)
            scalar.activation(
                out=sq_t[:, 2 * FEAT : 3 * FEAT],
                in_=d_t[1][:, 0:FEAT],
                func=mybir.ActivationFunctionType.Square,
                bias=res_t[:, BIAS_COL : BIAS_COL + 1],
                accum_out=res_t[:, 2:3],
            ).then_inc(s_qa, 1)

    # Strip the (now-unreferenced) const-AP registration memsets from the
    # preamble, and with them the entry all-engine barrier that only existed
    # to order those memsets before engine use. Together they delay every
    # engine's first instruction by ~0.8us. Barrier sems are self-resetting
    # (Pool writes them back to 0), so the exit barrier is unaffected.
    const_names = {"const-float32-0.0", "const-float32-1.0",
                   "const-bfloat16-1.0", "const-uint8-127"}
    for blk in nc.m.functions[0].blocks:
        if blk.name != "main":
            continue
        keep = []
        for ins in blk.instructions:
            tn = type(ins).__name__
            if tn == "InstMemset":
                tname = None
                try:
                    tname = ins.outs[0].bass_ap.tensor.name
                except Exception:
                    pass
                if tname in const_names:
                    continue
            if tn == "InstDrain":
                continue
            if tn == "InstEventSemaphore" and ins.name.startswith("barrier_"):
                continue
            keep.append(ins)
        if len(keep) != len(blk.instructions):
            blk.instructions[:] = keep

    nc.compile()
    return nc


def _host_inputs(x, centers, labels_i32):
    """Per-core input dicts."""
    # Scatter identity: unwrapped index k lives at [k % 16, k // 16];
    # replicate the 16-partition block across all 128 partitions.
    sidx16 = (
        np.arange(8, dtype=np.int16)[None, :] * 16
        + np.arange(16, dtype=np.int16)[:, None]
    )  # [16, 8], sidx16[ch, s] = s*16 + ch
    sidx = np.ascontiguousarray(np.tile(sidx16, (8, 1)))  # [128, 8]

    in_maps = []
    for c in range(N_CORES):
        sl = slice(c * SHARD, (c + 1) * SHARD)
        in_maps.append(
            {
                "centers": centers,
                "x": x[sl].reshape(P, SPP * FEAT),
                "gidx": np.ascontiguousarray(labels_i32[sl].reshape(P, SPP)),
                "sidx": sidx,
            }
        )
    return in_maps


def kernel(x, centers, labels):
    global _cached_nc, LAST_RESULT
    if _cached_nc is None:
        _cached_nc = _build_nc()
    nc = _cached_nc

    x = np.ascontiguousarray(x, dtype=np.float32)
    centers = np.ascontiguousarray(centers, dtype=np.float32)
    labels_i32 = np.ascontiguousarray(labels.astype(np.int32))

    in_maps = _host_inputs(x, centers, labels_i32)

    try:
        LAST_RESULT = run_bass_kernel_spmd(nc, in_maps, core_ids=list(range(N_CORES)))
    except ModuleNotFoundError:
        # BASS_TRACE=1 under axon needs antenv.axon_hooks, which some
        # containers lack; fall back to an untraced run instead of crashing.
        os.environ["BASS_NEVER_TRACE"] = "1"
        LAST_RESULT = run_bass_kernel_spmd(nc, in_maps, core_ids=list(range(N_CORES)))

    # res layout: col0 = s0, col1+col5 = s1 (split accum), col2 = s2,
    # col3 = s3; partition p holds samples 4p+0..3.
    dists = []
    for r in LAST_RESULT.results:
        o = r["out"]
        per = np.stack(
            [o[:, 0], o[:, 1] + o[:, S1B_COL], o[:, 2], o[:, 3]], axis=1
        )  # [128, 4]
        dists.append(per.reshape(-1))
    dist = np.concatenate(dists)
    dist = np.clip(dist, CLAMP_MIN, CLAMP_MAX)
    return np.asarray(dist.mean(dtype=np.float64), dtype=np.float32)
